# revision 16
# baseline (speedup 1.0000x reference)
"""Trainium2 Bass kernel for the DND memory-read module.

Per-sample computation (reference):
    A[t, n]   = (keys[t] * rpe[t]) . query[n]        (contract DK=128)
    w         = softmax_t(A)
    res[n, v] = sum_t w[t, n] * vals[t, v]           (contract T)
    out       = vec(res) @ W.T + b

Strategy: shard batch B=1024 across 8 cores (128 samples each). The
kernel is HBM-bound (~53 MB/core): keys fp16 with the rpe modulation
folded in on the host, vals fp8e3m4 with per-(t,b)-row scales folded
into the softmax weights. fp8e3m4 V feeds the PE directly as the
stationary operand of the res matmuls (mixed fp8 x fp16 matmul is
exact on TRN2; measured end-to-end max-rel ~1.1e-2 vs the 2e-2 gate),
which removes the 33M-element int8->fp16 dequant casts that saturated
DVE+ACT and consumer-paced the DMA rings in the int8 variant.

Schedule (v4): the Tile scheduler list-schedules per engine from a
CoreSim timeline that underestimates DMA ring FIFO latency, so any
program shape where late-arriving K tiles can be ordered ahead of
ready res work parks the in-order PE (measured 15us dead zones). The
DMA rings therefore carry strictly [qT, K0..K3, V0..V3]: during the K
phase the PE's only work IS the A/softmax chain, and during the V
phase res matmuls consume tiles in arrival order — there is no
ordering freedom left to lose. All softmax weights for every group are
ready before the first V tile lands. Bulk triggers are pre-issued
(sync HWDGE 5/8 : gpsimd SWDGE 3/8) with deep pools (K 14 x 0.5MB,
V 22 x 0.5MB) so the rings free-run at ~350 GB/s; output rows are
copied + DMA'd per group.

Per-core mapping (groups of 32 samples; rows (j, n) = sample-in-group x
head fill 128 partitions):
  A:    stationary = K_b^T chunk [d, t_chunk], mover = q_b^T [d, 4]
        -> psum [t_chunk, (c, j, n)] free-packed.
  A^T:  PE fp32 transpose -> [(j, n), t] rows for the softmax.
  softmax: DVE reduce_max(neg) + ACT exp with fused row-sum, DVE
        reciprocal + normalize + dequant-scale fold; weights fp16.
  w^T:  PE fp16 transpose back to [t, (j, n)].
  res:  stationary fp8 V_b chunk [t_chunk, v_chunk] straight from the
        dma tile, mover = w_b [t, 4] -> psum resT [v_sub, (vc, j, n)]
        — already transposed for the projection.
  out:  16 accumulating matmuls vec(res) @ W^T (+ bias via a K=1
        matmul), copied + DMA'd out per group.
"""

import numpy as np
import ml_dtypes

import concourse.bass as bass
import concourse.tile as tile
from concourse import mybir
from concourse.masks import make_identity


# ---------------------------------------------------------------------------
# Workaround: this walrus build rejects instructions with >2 sync commands.
# Tile's kernel-tail emits ONE drain on SP waiting on the whole global
# vector clock. Split those waits across a chain of drains (sequential
# waits == conjunction).
# ---------------------------------------------------------------------------
def _apply_tile_drain_patch():
    from concourse.vector_clock import ScopedClock, VectorClock

    def _drain_and_barrier_split(self, tick_clock, wait_clock):
        g = tick_clock.global_clock
        n = len(g)
        per = 1
        for i in range(0, n, per):
            vc = VectorClock([g[p] if i <= p < i + per else 0 for p in range(n)])
            d = self.nc.sync.drain()
            wait_clock.add_sem_waits(d.ins, ScopedClock({None: vc}))

        self.nc.all_engine_barrier()
        assert self.sems is not None
        popped = self.nc._tile_sem_poison_stack.pop()
        assert popped is self._sem_poison
        self.nc.clear_and_free_semaphores(list(self.sems.allocated().values()))
        self.nc.all_engine_barrier()

    tile.TileContext._drain_and_barrier = _drain_and_barrier_split


_apply_tile_drain_patch()


def _legalize_sync(nc, max_waits=1):
    """This walrus build allows very few sync commands per instruction.
    Keep at most one wait on each instruction; move overflow waits onto
    preceding same-engine NoOps, one wait per NoOp (engine executes them
    in order, so sequential waits == conjunction)."""
    for fn in nc.m.functions:
        for blk in fn.blocks:
            new_insts = []
            for inst in blk.instructions:
                si = inst.sync_info
                if si is not None:
                    waits = list(si.on_wait or [])
                    ups = list(si.on_update or [])
                    if len(waits) > max_waits:
                        extra = waits[:len(waits) - max_waits]
                        keep = waits[len(waits) - max_waits:]
                        for w in extra:
                            new_insts.append(mybir.InstNoOp(
                                name=f"legwait-{nc.next_id()}",
                                engine=inst.engine,
                                sync_info=mybir.SyncInfo(
                                    on_wait=[w], on_update=[]),
                            ))
                        inst.sync_info = mybir.SyncInfo(
                            on_wait=keep, on_update=ups)
                new_insts.append(inst)
            try:
                blk.instructions = new_insts
            except Exception:
                blk.instructions.clear()
                blk.instructions.extend(new_insts)


F16 = mybir.dt.float16
F32 = mybir.dt.float32
F8 = mybir.dt.float8e3


def build_core_program(B_l: int, m: int, NH: int = 4, DK: int = 128, V: int = 512,
                       OUT: int = 512, legalize: bool = True):
    """Build the single-core Bass program (SPMD: every core runs this)."""
    GS = 32                      # samples per group (GS*NH = 128 partitions)
    assert B_l % GS == 0
    G = B_l // GS                # groups
    m_pad = ((m + 127) // 128) * 128
    nch = m_pad // 128           # t-chunks
    NV = NH * V                  # flattened (n, v) contraction dim
    assert NV % 128 == 0
    nchw = NV // 128             # W^T chunks
    nvc = V // 128               # v-chunks
    OCT = 4                      # samples per K dma tile (0.5 MB)
    KTPG = GS // OCT             # K tiles per group (8)
    VCT = 8                      # samples per V dma tile (0.5 MB fp8)
    nu = GS // VCT               # V sample-chunks per group (4)
    VTPG = nch * nu              # V tiles per group
    full = (m == m_pad)

    nc = bass.Bass("TRN2")
    kT = nc.dram_tensor("kT", (DK, B_l, m_pad), F16, kind="ExternalInput")
    v4 = nc.dram_tensor("v4", (nch, 128, B_l, V), F8, kind="ExternalInput")
    vsc = nc.dram_tensor("vsc", (128, G, m_pad), F16, kind="ExternalInput")
    qT = nc.dram_tensor("qT", (DK, B_l * NH), F16, kind="ExternalInput")
    wT = nc.dram_tensor("wT", (128, nchw, OUT), F16, kind="ExternalInput")
    bias = nc.dram_tensor("bias", (1, OUT), F16, kind="ExternalInput")
    out = nc.dram_tensor("out", (B_l, OUT), F32, kind="ExternalOutput")

    with tile.TileContext(nc) as tc:
        with (
            tc.tile_pool(name="consts", bufs=1) as consts,
            tc.tile_pool(name="kpool", bufs=8) as kpool,
            tc.tile_pool(name="v8pool", bufs=28) as v8pool,
            tc.tile_pool(name="work", bufs=2) as work,
            tc.tile_pool(name="wtrs", bufs=max(4, G)) as wtrs,
            tc.tile_pool(name="stats", bufs=4) as stats,
            tc.tile_pool(name="pA", bufs=2, space="PSUM") as pA,
            tc.tile_pool(name="ptr", bufs=2, space="PSUM") as ptr,
            tc.tile_pool(name="presT", bufs=2, space="PSUM") as presT,
            tc.tile_pool(name="pout", bufs=1, space="PSUM") as pout,
        ):
            # ---- bulk DMA pre-issue ----------------------------------
            # Ring trigger order == consumption order == [K all, V all];
            # tiles split 5:3 across the sync (HWDGE, cheap triggers) and
            # gpsimd (SWDGE, ~1us/trigger) rings. Recycle semaphores pace
            # the hardware; compute engines trigger no bulk DMA.
            ring = [0]
            kt_tiles = []
            v8_tiles = []

            def ring_eng():
                r = ring[0] % 8
                ring[0] += 1
                return nc.sync if r in (0, 2, 3, 5, 6) else nc.gpsimd

            vno = [0]

            def ring_eng3():
                # The first 16 V tiles ride the otherwise-idle scalar
                # (ACT) HWDGE ring: their triggers are never recycle-
                # gated, so they stream DURING the K phase while the
                # sync/gpsimd rings are busy with K — res(0) can start
                # ~30us early. Later V tiles split 5:3 sync/gpsimd;
                # keeping them off ACT avoids recycle-gated triggers
                # head-of-line-blocking ACT's in-order queue.
                i = vno[0]
                vno[0] += 1
                if i < 16:
                    return nc.scalar
                return ring_eng()

            qT_sb = consts.tile([DK, B_l * NH], F16)
            nc.sync.dma_start(out=qT_sb, in_=qT[:, :])
            vsc_sb = consts.tile([128, G, m_pad], F16)
            nc.scalar.dma_start(out=vsc_sb, in_=vsc[:, :, :])
            bias_sb = consts.tile([1, OUT], F16)
            nc.scalar.dma_start(out=bias_sb, in_=bias[:, :])
            wT_sb = consts.tile([128, nchw, OUT], F16)
            nc.scalar.dma_start(out=wT_sb, in_=wT[:, :, :])
            # Ring order [K all, V all]: during the K phase the PE's only
            # work IS the A/softmax chain, so no scheduler reordering can
            # park the in-order PE on a far-future tile.
            for g in range(G):
                for o in range(KTPG):
                    t = kpool.tile([DK, OCT, m_pad], F16, tag="kt")
                    b0 = g * GS + o * OCT
                    ring_eng().dma_start(out=t, in_=kT[:, b0:b0 + OCT, :])
                    kt_tiles.append(t)
            for g in range(G):
                for c in range(nch):
                    for u in range(nu):
                        t = v8pool.tile([128, VCT, V], F8, tag="v8t")
                        b0 = g * GS + u * VCT
                        ring_eng3().dma_start(out=t, in_=v4[c, :, b0:b0 + VCT, :])
                        v8_tiles.append(t)
            ones_sb = consts.tile([1, 128], F16)
            nc.vector.memset(ones_sb, 1.0)
            ident16 = consts.tile([128, 128], F16)
            make_identity(nc, ident16)
            ident32 = consts.tile([128, 128], F32)
            make_identity(nc, ident32)
            out_ps = pout.tile([128, OUT], F32)

            def a_mm(g):
                """A matmuls -> psum A [t_sub, (c, j, n)]."""
                A_ps = pA.tile([128, nch, GS * NH], F32)
                for o in range(KTPG):
                    kt = kt_tiles[g * KTPG + o]
                    for j in range(OCT):
                        b = g * GS + o * OCT + j
                        js = (o * OCT + j) * NH
                        for c in range(nch):
                            # ONE accumulation group per psum bank: start
                            # invalidates the whole 2KB zero region, so
                            # only the first matmul may carry it.
                            nc.tensor.matmul(
                                A_ps[:, c, js:js + NH],
                                kt[:, j, c * 128:(c + 1) * 128],
                                qT_sb[:, b * NH:(b + 1) * NH],
                                start=(o == 0 and j == 0 and c == 0),
                                stop=(o == KTPG - 1 and j == OCT - 1
                                      and c == nch - 1),
                            )
                return A_ps

            def sm_phase(g, A_ps):
                """Transpose A, softmax rows, transpose weights back."""
                AT_sb = work.tile([128, nch, GS * NH], F32, tag="atsb")
                nc.scalar.activation(out=AT_sb, in_=A_ps,
                                     func=mybir.ActivationFunctionType.Copy)
                A2_ps = ptr.tile([128, nch * 128], F32, tag="ptr")
                for c in range(nch):
                    nc.tensor.matmul(A2_ps[:, c * 128:(c + 1) * 128],
                                     AT_sb[:, c, :], ident32,
                                     is_transpose=True,
                                     start=(c == 0), stop=(c == nch - 1))
                # rpe is folded into K on the host, so reduce_max and exp
                # read the transposed-A psum directly.
                negmax = stats.tile([128, 1], F32, tag="negmax")
                nc.vector.reduce_max(negmax, A2_ps[:, :m],
                                     axis=mybir.AxisListType.X, negate=True)
                wt = work.tile([128, m_pad], F16, tag="wt")
                if not full:
                    nc.vector.memset(wt, 0.0)
                ssum = stats.tile([128, 1], F32, tag="ssum")
                nc.scalar.activation(
                    out=wt[:, :m], in_=A2_ps[:, :m],
                    func=mybir.ActivationFunctionType.Exp,
                    bias=negmax, scale=1.0, accum_out=ssum,
                )
                rinv = stats.tile([128, 1], F32, tag="rinv")
                nc.vector.reciprocal(rinv, ssum)
                wn0 = work.tile([128, m_pad], F16, tag="wn0")
                nc.vector.tensor_scalar_mul(wn0, in0=wt, scalar1=rinv)
                # fold the per-(t, b) fp8 dequant scales of V into the
                # normalized weights: res = sum_t (w*s) * V8
                wn = work.tile([128, m_pad], F16, tag="wn")
                nc.vector.tensor_mul(wn, wn0, vsc_sb[:, g, :])
                ptr_w = ptr.tile([128, nch * 128], F16, tag="ptr")
                for c in range(nch):
                    nc.tensor.matmul(ptr_w[:, c * 128:(c + 1) * 128],
                                     wn[:, c * 128:(c + 1) * 128], ident16,
                                     is_transpose=True,
                                     start=(c == 0), stop=(c == nch - 1))
                wTr = wtrs.tile([128, nch, 128], F16, tag="wtr")
                nc.vector.tensor_copy(
                    wTr, ptr_w.rearrange("p (c t) -> p c t", c=nch))
                return wTr

            def res_mm(g, wTr):
                """res matmuls straight off the fp8 dma tiles (stationary
                = fp8 V chunk, mover = fp16 folded weights; the per-row
                dequant scale rides in the weights). Ends with the
                psum->sbuf copy (ACT) so it can drain while the PE moves
                on to the next group's res matmuls."""
                rT_ps = presT.tile([128, nvc, GS * NH], F32)
                for c in range(nch):
                    for u in range(nu):
                        v8t = v8_tiles[g * VTPG + c * nu + u]
                        for j in range(VCT):
                            js = (u * VCT + j) * NH
                            for vc in range(nvc):
                                nc.tensor.matmul(
                                    rT_ps[:, vc, js:js + NH],
                                    v8t[:, j, vc * 128:(vc + 1) * 128],
                                    wTr[:, c, js:js + NH],
                                    start=(c == 0 and u == 0 and j == 0
                                           and vc == 0),
                                    stop=(c == nch - 1 and u == nu - 1
                                          and j == VCT - 1 and vc == nvc - 1),
                                )
                # psum->sbuf copy on DVE (idle once the softmaxes are
                # done) so ACT's in-order queue stays pure trigger/exp.
                resT_g = work.tile([128, nvc, GS * NH], F16, tag="resTg")
                nc.vector.tensor_copy(resT_g, rT_ps)
                return resT_g

            def proj(g, resT_g):
                """out[g*32:(g+1)*32] = vec(res) @ W^T + b, DMA'd out."""
                ob = g * GS
                nc.tensor.matmul(out_ps[ob:ob + GS, :], ones_sb[:, :GS],
                                 bias_sb, start=True, stop=False,
                                 tile_position=(0, ob))
                for n in range(NH):
                    for vc in range(nvc):
                        cp = n * nvc + vc
                        lhsT = resT_g[:, vc, :].rearrange(
                            "p (b n) -> p n b", n=NH)[:, n, :]
                        nc.tensor.matmul(
                            out_ps[ob:ob + GS, :], lhsT, wT_sb[:, cp, :],
                            start=False, stop=(cp == nchw - 1),
                            tile_position=(0, ob),
                        )
                out_g = work.tile([GS, OUT], F32, tag="outg")
                nc.vector.tensor_copy(out_g, out_ps[ob:ob + GS, :])
                nc.sync.dma_start(out=out[ob:ob + GS, :], in_=out_g)

            # ---- group pipeline --------------------------------------
            # All A/softmax chains first (they track the K stream), then
            # the res phases track the V stream tile by tile. proj(g) is
            # deferred until after res(g+1) so the in-order PE never
            # parks on the resT psum->sbuf copy latency.
            wTr_list = []
            for g in range(G):
                A_ps = a_mm(g)
                wTr_list.append(sm_phase(g, A_ps))
            resT_prev = None
            for g in range(G):
                resT_cur = res_mm(g, wTr_list[g])
                if resT_prev is not None:
                    proj(g - 1, resT_prev)
                resT_prev = resT_cur
            proj(G - 1, resT_prev)

    if legalize:
        _legalize_sync(nc)
    return nc


def prep_core_inputs(keys, vals, rpe, query, W, b, m, n_cores=8):
    """Host-side shard + relayout + fp16/fp8 cast. Returns list of in_maps."""
    T, B, DK = keys.shape
    V = vals.shape[2]
    NH = query.shape[1]
    OUT = W.shape[0]
    B_l = B // n_cores
    m_pad = ((m + 127) // 128) * 128
    nch = m_pad // 128
    G = B_l // 32

    keys = keys[:m]
    vals = vals[:m]
    rpe = rpe[:m]

    # keys^T: [T,B,DK] -> fp16 [DK, B, m_pad], with the rpe modulation
    # folded in on the host (f32 multiply, single fp16 rounding)
    kT = np.zeros((DK, B, m_pad), np.float16)
    kT[:, :, :m] = (keys * rpe).transpose(2, 1, 0)
    # vals: [T,B,V] -> fp8e3m4 [nch, 128, B, V] with per-(t,b)-row scales
    # (max maps to 15.5, the e3m4 max normal)
    vmax = np.abs(vals).max(axis=2, keepdims=True)          # [m, B, 1]
    vs = np.maximum(vmax / 15.5, 1e-20).astype(np.float32)
    v4 = np.zeros((nch, 128, B, V), ml_dtypes.float8_e3m4)
    vq = vals / vs
    np.clip(vq, -15.5, 15.5, out=vq)
    v4.reshape(m_pad, B, V)[:m] = vq.astype(ml_dtypes.float8_e3m4)
    del vq
    # dequant scales: [m, B] -> fp16 [B, m_pad] row layout
    sc = np.zeros((B, m_pad), np.float16)
    sc[:, :m] = vs[:, :, 0].T
    # qT: [B,NH,DK] -> fp16 [DK, B*NH]
    qTf = query.transpose(2, 0, 1).reshape(DK, B * NH).astype(np.float16)
    # W^T: [OUT, NH*V] -> fp16 [128, nchw, OUT]
    nchw = (NH * V) // 128
    wTf = np.ascontiguousarray(
        W.T.reshape(nchw, 128, OUT).transpose(1, 0, 2)).astype(np.float16)
    biasf = b.reshape(1, OUT).astype(np.float16)

    def row_layout(mat, bs):
        """[B, m_pad] -> per-core [128 rows=(j,n), G, m_pad]."""
        x = mat[bs].reshape(G, 32, m_pad)
        x = np.repeat(x, NH, axis=1)                   # [G, 128, m_pad]
        return np.ascontiguousarray(x.transpose(1, 0, 2))

    in_maps = []
    for c in range(n_cores):
        bs = slice(c * B_l, (c + 1) * B_l)
        in_maps.append({
            "kT": np.ascontiguousarray(kT[:, bs, :]),
            "v4": np.ascontiguousarray(v4[:, :, bs, :]),
            "vsc": row_layout(sc, bs),
            "qT": np.ascontiguousarray(
                qTf.reshape(DK, B, NH)[:, bs, :].reshape(DK, B_l * NH)),
            "wT": wTf,
            "bias": biasf,
        })
    return in_maps


def kernel(keys_mem, vals_mem, rpe, query, W, b, min_step):
    from concourse import bass_utils

    keys_mem = np.asarray(keys_mem, dtype=np.float32)
    vals_mem = np.asarray(vals_mem, dtype=np.float32)
    rpe = np.asarray(rpe, dtype=np.float32)
    query = np.asarray(query, dtype=np.float32)
    W = np.asarray(W, dtype=np.float32)
    b = np.asarray(b, dtype=np.float32)
    m = int(min_step)

    n_cores = 8
    T, B, DK = keys_mem.shape
    B_l = B // n_cores

    nc = build_core_program(B_l, m, NH=query.shape[1], DK=DK,
                            V=vals_mem.shape[2], OUT=W.shape[0])
    in_maps = prep_core_inputs(keys_mem, vals_mem, rpe, query, W, b, m,
                               n_cores=n_cores)
    res = bass_utils.run_bass_kernel_spmd(nc, in_maps,
                                          core_ids=list(range(n_cores)))
    return np.concatenate([res.results[c]["out"] for c in range(n_cores)],
                          axis=0)
